# revision 79
# baseline (speedup 1.0000x reference)
"""Additive (Bahdanau) attention on 4 of 8 TRN2 NeuronCores.

Problem shapes: B=4, Q=512, K=1024, Dq=Dk=Dv=512, H=128.

Sharding: one batch per core on a 4-core mesh (cores 4-7 idle). The
metric this kernel is tuned for is the MARGINAL WALL-CLOCK PER DISPATCH
through the axon tunnel, and that cost scales with mesh size (~100 us
fixed + ~15-30 us per core: measured sustained slopes for a trivial
NEFF are 99/92/116/233 us at 1/2/4/8 cores), while the on-device time
scales down with more cores. 8 cores put the device at ~72 us but
dispatch at ~233 us; 4 cores put dispatch at ~116 us and the device at
~75 us (each core shares ALL key-side work -- k features, k trig, the
values load -- across its four query blocks). The dispatch cost also
grows with total buffer bytes, which is why the packed input ships as
bf16 (2.9 MB instead of 5.8 MB; everything is consumed as bf16
on-chip anyway).

Algorithm (sine decomposition of additive attention):

tanh(a+b) is separable through the angle-addition identity. Fit
tanh(x) ~ sum_r c_r sin(w_r x) (weighted least squares, R=7, wmax=3.0,
core max err ~8e-3 over the feature-sum range; softmax and the w_v
weighting absorb most of it -- measured output rel err is 2.7e-3 vs
the 2e-2 gate), then

  scores[q,k] = sum_h w_h tanh(qf_hq + kf_hk)
             = sum_r [ (c_r w_h sin(w_r qf)) . cos(w_r kf)
                     + (c_r w_h cos(w_r qf)) . sin(w_r kf) ]   (contract h)

i.e. 2R=14 accumulating 128-contraction matmuls on the tensor engine
instead of 268M scalar-engine tanh evaluations (~300 us/core direct).
The ACT Sin table is only accurate on [-pi, pi], so arguments are
range-reduced exactly, with both sides of the problem fused into wide
per-r ops over the combined [qf | kf] feature tile (see the inline
comments in _build_sine for the exact op chain).

IMPORTANT: no GPSIMD (Q7) instructions anywhere -- each dispatch of a
NEFF containing GPSIMD ops costs ~300-400 us of extra per-dispatch
host/runtime overhead under axon, dwarfing the on-device time. The
trig chains are balanced across DVE and ACT only. sin/cos tiles are
bf16 (the c_r*w_v weighting is folded into the q-side, keeping
per-term magnitudes small). Inputs arrive as ONE packed DRAM
parameter (fewer PJRT buffer binds per dispatch).

Scores are computed in [q, k] orientation (one [128, 1024] 2-bank PSUM
region per 128-query block, a single accumulation group, so plain
matmul start/stop works with no dummy zeroing). Softmax needs no
max-subtraction (scores are O(1) by construction: w_v has variance
1/H); exp's accum_out gives the denominator for free, and the attn
tile is xbar-DMA-transposed into the stationary operand of the attn@V
matmuls. Walrus's s3d3_mm ISA check caps a matmul's moving fmap at 512
elements and the AP partition stride at 16 KiB/row -- the score
matmuls are split into 512-wide halves and the per-r sin/cos tiles
stay separate for these reasons.

Dispatch-path notes (the dominant cost): the runner compiles under
bass2jax._fast_dispatch_active(True) so _bass_exec_p declares no
effect and calls take jax's C++ pjit fast path (the Python
effects/token path costs ~0.3-0.7 ms/call more); it returns the raw
Compiled rather than the FastDispatchCompiled wrapper (whose per-call
Python re-registration of every output shard costs ~0.1-0.2 ms); and
it passes no zero-filled output operands (on the exec lowering path
the NEFF binds only BIR ExternalInputs, and this kernel DMA-writes
every element of out). partition_id is disabled (unused input).
"""


import os
import ml_dtypes
import numpy as np

import concourse.bass as bass
import concourse.mybir as mybir
import concourse.tile as tile
from concourse import bacc
from concourse.bass_utils import run_bass_kernel_spmd
from concourse.masks import make_identity

B, Q, K, D, H = 4, 512, 1024, 512, 128
N_CORES = 4
QSH = Q                         # 512 query rows per core (one batch/core)
QH = 256                        # query rows per PSUM score pass
NDC = D // 128                  # 4 contraction chunks
NKC = K // 128                  # 8 key chunks
NQB = QSH // 128                # 4 query blocks per core

F32 = mybir.dt.float32
BF16 = mybir.dt.bfloat16
EXP = mybir.ActivationFunctionType.Exp
SIN = mybir.ActivationFunctionType.Sin
COPY = mybir.ActivationFunctionType.Copy
TS = mybir.AluOpType

MAGIC = 12582912.0              # 1.5 * 2**23: fp32 add forces round-to-int
TWO_PI = float(2.0 * np.pi)

LAST_EXEC_NS = None
_NC_CACHE = {}


R_SINE = 7
WMAX = 3.0

def _fit_sine(R=R_SINE, wmax=WMAX, L=7.5, sigma=2.8):
    """Least-squares fit tanh(x) ~ sum_r c_r sin(w_r x) on [-L, L]."""
    ws = np.linspace(wmax / R * 0.5, wmax, R)
    xs = np.linspace(-L, L, 4001)
    wt = np.exp(-xs ** 2 / (2 * sigma ** 2)) + 1e-3
    A = np.sin(np.outer(xs, ws))
    Wt = np.sqrt(wt)[:, None]
    c, *_ = np.linalg.lstsq(A * Wt, np.tanh(xs) * Wt[:, 0], rcond=None)
    return [float(w) for w in ws], [float(v) for v in c]


# Packed single-input layout (one NEFF parameter instead of six: fewer PJRT
# buffer binds per dispatch, which dominates the per-call overhead under axon).
OFF_Q = 0
OFF_K = OFF_Q + QSH * D          # 262144
OFF_V = OFF_K + K * D            # 786432
OFF_WQ = OFF_V + K * D           # 1310720
OFF_WK = OFF_WQ + D * H          # 1376256
OFF_WV = OFF_WK + D * H          # 1441792
NPACK = OFF_WV + H               # 1441920


def _declare_io(nc):
    # The packed input is BF16: every tensor is consumed as bf16 on-chip
    # anyway (features, trig, attn@V), so shipping bf16 halves the input
    # DMA bus time AND lets the xbar DMA-transposes read q/k straight from
    # DRAM with no staging tiles, no casts, and no queue dependencies.
    # Only w_v is precision-sensitive enough to notice, and its bf16
    # rounding adds well under 1e-3 output rel err.
    xin = nc.declare_dram_parameter("xin", [NPACK], BF16, isOutput=False)
    # Output is also bf16 (kernel() upconverts to f32 on the host): the
    # per-dispatch tunnel cost scales with buffer bytes, and the output
    # values' bf16 rounding (<=0.4% per element) fits the error budget.
    out_ext = nc.declare_dram_parameter("out", [QSH, D], BF16, isOutput=True)
    q_ext = xin[OFF_Q:OFF_K].rearrange("(q d) -> q d", d=D)
    k_ext = xin[OFF_K:OFF_V].rearrange("(k d) -> k d", d=D)
    v_ext = xin[OFF_V:OFF_WQ].rearrange("(k d) -> k d", d=D)
    wq_ext = xin[OFF_WQ:OFF_WK].rearrange("(d h) -> d h", h=H)
    wk_ext = xin[OFF_WK:OFF_WV].rearrange("(d h) -> d h", h=H)
    wv_ext = xin[OFF_WV:NPACK].rearrange("(h o) -> h o", o=1)
    return q_ext, k_ext, v_ext, wq_ext, wk_ext, wv_ext, out_ext


def _preamble(nc, tc, const, work, feat, q_ext, k_ext, v_ext, wq_ext, wk_ext,
              wv_ext):
    """Load + transpose inputs, feature matmuls.

    The packed input is bf16, so the xbar DMA-transposes read q and k
    STRAIGHT FROM DRAM: no f32 staging tiles, no casts, and -- because a
    DRAM source has no producer -- no semaphore waits on the in-order SP
    queue. A DRAM [rows, 512] transpose lands as [128, NDC, rows]
    (logical row d -> partition d % 128, chunk d // 128), so the feature
    matmuls read contiguous [128, rows] moving fmaps. k is transposed in
    256-row quarters so each quarter's kf matmuls overlap the next
    quarter's transfer; each quarter accumulates in its own single-bank
    PSUM tile. v loads directly as the bf16 attn@V operand. Every
    per-DMA instruction costs ~0.6-1.2 us of sequencer issue + ~0.9 us
    of completion semaphore, so transfers are batched (8 DMA instructions
    total for 2.9 MB).

    Returns (fq [H, QSH+K] f32 ([qf | kf]), v_b [128, NKC, D] bf16,
    wv_f [H,1] f32)."""
    wq_b = const.tile([128, NDC, H], BF16)
    wk_b = const.tile([128, NDC, H], BF16)
    nc.sync.dma_start(out=wk_b, in_=wk_ext.rearrange("(c p) h -> p c h", p=128))
    nc.sync.dma_start(out=wq_b, in_=wq_ext.rearrange("(c p) h -> p c h", p=128))

    wv_b = const.tile([H, 1], BF16)
    nc.sync.dma_start(out=wv_b, in_=wv_ext[:])
    wv_f = const.tile([H, 1], F32)
    nc.vector.tensor_copy(wv_f, wv_b)

    qTb = work.tile([128, NDC, QSH], BF16)
    nc.sync.dma_start_transpose(out=qTb[:], in_=q_ext)
    kTb = work.tile([128, 4, NDC, 256], BF16)
    for qd in range(4):
        nc.sync.dma_start_transpose(out=kTb[:, qd],
                                    in_=k_ext[qd * 256:(qd + 1) * 256, :])

    # fq = [qf | kf] in ONE tile so the per-r trig argument prep can run as
    # single wide DVE ops over both sides at once.
    fq = feat.tile([H, QSH + K], F32)
    qf_sb = fq[:, :QSH]
    kf_sb = fq[:, QSH:]
    with tc.tile_pool(name="pre_ps", bufs=4, space="PSUM") as pre_ps:
        qf_ps = pre_ps.tile([H, QSH], F32, tag="fps")
        for dc in range(NDC):
            nc.tensor.matmul(qf_ps, wq_b[:, dc, :], qTb[:, dc, :],
                             start=(dc == 0), stop=(dc == NDC - 1))
        nc.vector.tensor_copy(qf_sb, qf_ps)

        for qd in range(4):
            kf_q = pre_ps.tile([H, 256], F32, tag="kfq")
            for dc in range(NDC):
                nc.tensor.matmul(kf_q, wk_b[:, dc, :], kTb[:, qd, dc, :],
                                 start=(dc == 0), stop=(dc == NDC - 1))
            dst = kf_sb[:, qd * 256:(qd + 1) * 256]
            (nc.vector.tensor_copy(dst, kf_q) if qd % 2 == 0
             else nc.scalar.copy(dst, kf_q))

    v_b = feat.tile([128, NKC, D], BF16)
    tc.tile_set_cur_wait(0.05)   # keep values off the keys->kf critical path
    nc.sync.dma_start(out=v_b,
                      in_=v_ext.rearrange("(c p) d -> p c d", p=128))
    tc.tile_set_cur_wait(0)

    return fq, v_b, wv_f


def _build_sine():
    ws, cs = _fit_sine()
    R = len(ws)
    nc = bacc.Bacc(enable_partition_id=False)
    q_ext, k_ext, v_ext, wq_ext, wk_ext, wv_ext, out_ext = _declare_io(nc)

    with tile.TileContext(nc) as tc:
        with tc.tile_pool(name="const", bufs=1) as const, \
             tc.tile_pool(name="feat", bufs=1) as feat, \
             tc.tile_pool(name="trig", bufs=4) as trig, \
             tc.tile_pool(name="oloop", bufs=2) as oloop:

            # The staging tiles (k_all/q_all/kT/qT/v_stage, ~64 KiB/
            # partition) live in their own pool that closes after the
            # preamble, freeing the space for the deeper (bufs=3) trig
            # pipeline.
            with tc.tile_pool(name="featpre", bufs=1) as featpre:
                fq, v_b, wv_f = _preamble(
                    nc, tc, const, featpre, feat, q_ext, k_ext, v_ext,
                    wq_ext, wk_ext, wv_ext)

            # per-r q-side coefficient vectors: wc[:, r] = c_r * w_v
            wc = const.tile([H, R], F32)
            for r in range(R):
                nc.vector.tensor_scalar_mul(wc[:, r:r + 1], wv_f, float(cs[r]))

            W2 = QSH + K                 # one trig block: [q(512) | k(1024)]
            # SCB[r] = [wc*sin_q(512) | sin_k(1024) |
            #           wc*cos_q(512) | cos_k(1024)]
            # One tile PER r (not [H, R, 2*W2]): the PE matmul AP encodes a
            # per-partition stride that maxes out at 16 KiB, and the fused
            # tile's 24 KiB row fails walrus's s3d3_mm ISA check.
            SCB = [feat.tile([H, 2 * W2], BF16, name=f"scb{r}")
                   for r in range(R)]

            # Per r, the range reduction is 3 wide DVE ops + 1 ACT round +
            # ONE double-wide ACT Sin over the combined [qf | kf] block:
            #   t   = x * (w/2pi)                  (DVE tensor_scalar)
            #   a   = fl(t + 1.5*2^23)             (ACT Copy, float bias:
            #                                       the fp32 store rounds t
            #                                       to the nearest integer)
            #   e_s = (a - MAGIC) - t              (DVE scalar_tensor_tensor,
            #                                       exact) = round(t) - t
            #   e_c = wrap(e_s - 1/4)              (custom-DVE range wrap
            #                                       into [-1/2, 1/2])
            #   sin/cos(w x) = Sin(-2pi * e)       (ONE ACT Sin over the
            #                                       contiguous [e_s | e_c]
            #                                       tile; table is accurate
            #                                       on [-pi, pi])
            # The q-side c_r*w_v weighting is applied in place on the bf16
            # sin/cos q slices (2 narrow DVE muls). No GPSIMD (Q7) anywhere:
            # those cost ~300us of per-dispatch host overhead under axon.
            for r in range(R):
                w2p = float(ws[r] / TWO_PI)
                wcol = wc[:, r:r + 1]
                t_t = trig.tile([H, W2], F32, tag="t")
                nc.vector.tensor_scalar(t_t, fq, w2p, None, TS.mult)
                a_t = trig.tile([H, W2], F32, tag="a")
                nc.scalar.activation(out=a_t, in_=t_t, func=COPY, bias=MAGIC)
                arg = trig.tile([H, 2 * W2], F32, tag="arg")
                nc.vector.scalar_tensor_tensor(arg[:, :W2], a_t, MAGIC, t_t,
                                               TS.subtract, TS.subtract)
                nc.vector.add_range_wrap(arg[:, W2:], arg[:, :W2],
                                         -0.25, 0.5, 1.0)
                nc.scalar.activation(out=SCB[r][:], in_=arg,
                                     func=SIN, scale=-TWO_PI)
                nc.vector.tensor_scalar_mul(SCB[r][:, 0:QSH],
                                            SCB[r][:, 0:QSH], wcol)
                nc.vector.tensor_scalar_mul(SCB[r][:, W2:W2 + QSH],
                                            SCB[r][:, W2:W2 + QSH], wcol)


            # Scores in [q, k] orientation: per 128-query block, stationary
            # = QS/QC q-chunk [H, 128] and moving = the FULL 1024-wide
            # KS/KC row, so one PSUM pass is 2R=16 matmuls of 1024 moving
            # cols instead of 128 matmuls of 256 cols. This unloads the PE
            # SEQUENCER (Ldweights+Matmult issue was the critical path at
            # 256 score matmuls). Each [128, 1024] f32 region is exactly 2
            # PSUM banks used by a single accumulation group, so plain
            # start/stop works -- no dummy-zero matmuls.
            #
            # attn is then transposed for the attn@V matmuls by ONE xbar
            # DMA-transpose per block (64 16x128 tiles, ~1 us on the idle
            # DMA engines) instead of 8 PE transposes + 8 PSUM copies. The
            # xbar writes logical row r of attn^T to attnT[r % 128,
            # r // 128, :] (hardware-verified), i.e. k-chunk c holds k rows
            # {c*128+p} -- the natural chunk layout v_b is loaded in.
            # Score matmuls for block qb+1 are issued BEFORE block qb's
            # transpose/AV so the PE never stalls on qb's exp.
            o_all = feat.tile([128, NQB, D], BF16)
            with tc.tile_pool(name="psqk", bufs=3, space="PSUM") as psqk, \
                 tc.tile_pool(name="ps", bufs=2, space="PSUM") as ps:
                qsc = [None] * NQB

                def issue_scores(qb):
                    qs_sl = slice(qb * 128, (qb + 1) * 128)
                    qc_sl = slice(W2 + qb * 128, W2 + (qb + 1) * 128)
                    ks_sl = slice(QSH, W2)
                    kc_sl = slice(W2 + QSH, 2 * W2)
                    qsc[qb] = psqk.tile([128, K], F32, tag="qsc",
                                        name=f"qsc{qb}")
                    # moving operands are split into 512-wide halves: a
                    # 1024-element moving fmap fails walrus's s3d3_mm ISA
                    # check. The stationary is identical for both halves, so
                    # the second matmul skips its Ldweights.
                    for r in range(R):
                        for hf in range(2):
                            osl = slice(hf * 512, (hf + 1) * 512)
                            nc.tensor.matmul(
                                qsc[qb][:, osl], SCB[r][:, qs_sl],
                                SCB[r][:, kc_sl][:, osl],
                                start=(r == 0), stop=False,
                                skip_group_check=True)
                        for hf in range(2):
                            osl = slice(hf * 512, (hf + 1) * 512)
                            nc.tensor.matmul(
                                qsc[qb][:, osl], SCB[r][:, qc_sl],
                                SCB[r][:, ks_sl][:, osl],
                                start=False, stop=(r == R - 1),
                                skip_group_check=True)

                issue_scores(0)
                issue_scores(1)
                for qb in range(NQB):
                    attnQ = oloop.tile([128, K], BF16, tag="attnQ")
                    d_sb = oloop.tile([128, 1], F32, tag="dsb")
                    # accum_out gives the softmax denominator for free: in
                    # the [q, k] orientation the activation's per-partition
                    # output sum IS sum_k exp(score[q, k]).
                    nc.scalar.activation(out=attnQ, in_=qsc[qb], func=EXP,
                                         accum_out=d_sb)
                    if qb + 2 < NQB:
                        issue_scores(qb + 2)
                    attnT = oloop.tile([128, NKC, 128], BF16, tag="attnT")
                    nc.sync.dma_start_transpose(out=attnT[:], in_=attnQ[:])
                    o_ps = ps.tile([128, D], F32, tag="ops")
                    for kc in range(NKC):
                        nc.tensor.matmul(o_ps, attnT[:, kc, :], v_b[:, kc, :],
                                         start=(kc == 0), stop=(kc == NKC - 1))
                    recip = oloop.tile([128, 1], F32, tag="recip")
                    nc.vector.reciprocal(recip, d_sb)
                    nc.vector.tensor_scalar_mul(o_all[:, qb, :], o_ps, recip)
                    if qb == 1:
                        # first output half leaves while qb2/qb3 compute
                        nc.sync.dma_start(
                            out=out_ext.rearrange(
                                "(t p) d -> p t d", p=128)[:, 0:2, :],
                            in_=o_all[:, 0:2, :])
                nc.sync.dma_start(
                    out=out_ext.rearrange("(t p) d -> p t d", p=128)[:, 2:4, :],
                    in_=o_all[:, 2:4, :])
    nc.compile()
    return nc


def _get_nc():
    if "sine" not in _NC_CACHE:
        _NC_CACHE["sine"] = _build_sine()
    return _NC_CACHE["sine"]


def make_in_maps(queries, keys, values, W_q, W_k, w_v):
    bf16 = ml_dtypes.bfloat16
    queries = np.asarray(queries).astype(bf16)
    keys = np.asarray(keys).astype(bf16)
    values = np.asarray(values).astype(bf16)
    W_q = np.asarray(W_q).astype(bf16).ravel()
    W_k = np.asarray(W_k).astype(bf16).ravel()
    w_v = np.asarray(w_v).astype(bf16).ravel()
    in_maps = []
    for c in range(N_CORES):
        buf = np.empty(NPACK, bf16)
        buf[OFF_Q:OFF_K] = queries[c].ravel()
        buf[OFF_K:OFF_V] = keys[c].ravel()
        buf[OFF_V:OFF_WQ] = values[c].ravel()
        buf[OFF_WQ:OFF_WK] = W_q
        buf[OFF_WK:OFF_WV] = W_k
        buf[OFF_WV:NPACK] = w_v
        in_maps.append({"xin": buf})
    return in_maps


_RUNNER_CACHE = {}


def _get_runner(nc):
    """Persistent compiled shard_map runner for nc (compiled once/process).

    Two dispatch-path choices matter for the marginal per-call cost under
    axon (the per-dispatch host overhead dominates on-device time):

    * compile under bass2jax._fast_dispatch_active(True): _bass_exec_p then
      declares no effect, so calls take jax's C++ pjit fast path instead of
      the Python effects/token dispatch (~0.3-0.7 ms/call cheaper).
    * return the raw Compiled, NOT FastDispatchCompiled: the safety-net
      wrapper re-registers every output shard in runtime_tokens on every
      call (a Python loop over the shards, ~0.1-0.2 ms/call). kernel()
      reads its outputs immediately, so device errors surface regardless.
    * no zero-filled output operands: on the exec lowering path the NEFF
      binds only BIR ExternalInputs (the "out" zeros param has no NEFF
      tensor and is ignored), and this kernel DMA-writes every element of
      out, so PJRT's uninitialized result allocation is fine. Dropping
      them saves one buffer bind per core per call.
    """
    if id(nc) in _RUNNER_CACHE:
        return _RUNNER_CACHE[id(nc)]
    import jax
    from jax.sharding import Mesh, NamedSharding, PartitionSpec
    from jax.experimental.shard_map import shard_map
    from concourse import bass2jax

    bass2jax.install_neuronx_cc_hook()
    partition_name = (nc.partition_id_tensor.name
                      if nc.partition_id_tensor else None)
    in_names, in_shapes, out_names, out_avals = [], [], [], []
    for alloc in nc.m.functions[0].allocations:
        if not isinstance(alloc, mybir.MemoryLocationSet):
            continue
        name = alloc.memorylocations[0].name
        if alloc.kind == "ExternalInput":
            if name != partition_name:
                in_names.append(name)
                in_shapes.append(
                    (tuple(alloc.tensor_shape), mybir.dt.np(alloc.dtype)))
        elif alloc.kind == "ExternalOutput":
            out_names.append(name)
            shape = tuple(alloc.tensor_shape)
            dtype = mybir.dt.np(alloc.dtype)
            out_avals.append(jax.core.ShapedArray(shape, dtype))
    all_in_names = list(in_names)
    if partition_name is not None:
        all_in_names.append(partition_name)

    def _body(*args):
        operands = list(args)
        if partition_name is not None:
            operands.append(bass2jax.partition_id_tensor())
        outs = bass2jax._bass_exec_p.bind(
            *operands,
            out_avals=tuple(out_avals),
            in_names=tuple(all_in_names),
            out_names=tuple(out_names),
            lowering_input_output_aliases=(),
            sim_require_finite=True,
            sim_require_nnan=True,
            nc=nc,
        )
        return tuple(outs)

    devices = jax.devices()[:N_CORES]
    mesh = Mesh(np.asarray(devices), ("core",))
    nio = len(in_names)
    sharding = NamedSharding(mesh, PartitionSpec("core"))
    fast_ctx = getattr(bass2jax, "_fast_dispatch_active", None)
    import contextlib
    with (fast_ctx(True) if fast_ctx is not None
          else contextlib.nullcontext()):
        f = jax.jit(
            shard_map(_body, mesh=mesh,
                      in_specs=(PartitionSpec("core"),) * nio,
                      out_specs=(PartitionSpec("core"),) * len(out_names),
                      check_rep=False),
            keep_unused=True,
        )
        dummy_in = [
            jax.ShapeDtypeStruct((N_CORES * shape[0], *shape[1:]), dtype,
                                 sharding=sharding)
            for shape, dtype in in_shapes
        ]
        fc = f.lower(*dummy_in).compile()
    runner = (fc, in_names, out_names, out_avals, sharding)
    _RUNNER_CACHE[id(nc)] = runner
    return runner


def kernel(queries, keys, values, W_q, W_k, w_v):
    import jax
    nc = _get_nc()
    in_maps = make_in_maps(queries, keys, values, W_q, W_k, w_v)
    try:
        fc, in_names, out_names, out_avals, sharding = _get_runner(nc)
        concat_in = [
            np.concatenate([in_maps[c][name] for c in range(N_CORES)], axis=0)
            for name in in_names
        ]
        args = [jax.device_put(a, sharding) for a in concat_in]
        out_arrs = fc(*args)
        results = [
            {name: np.asarray(out_arrs[i]).reshape(
                N_CORES, *out_avals[i].shape)[c]
             for i, name in enumerate(out_names)}
            for c in range(N_CORES)
        ]
    except Exception:
        res = run_bass_kernel_spmd(nc, in_maps, core_ids=list(range(N_CORES)))
        results = res.results

    out = np.empty((B, Q, D), dtype=np.float32)
    for c in range(N_CORES):
        out[c] = results[c]["out"].astype(np.float32)
    return out


# revision 81
# speedup vs baseline: 1.0990x; 1.0990x over previous
"""Additive (Bahdanau) attention on 4 of 8 TRN2 NeuronCores.

Problem shapes: B=4, Q=512, K=1024, Dq=Dk=Dv=512, H=128.

Sharding: one batch per core on a 4-core mesh (cores 4-7 idle). The
metric this kernel is tuned for is the MARGINAL WALL-CLOCK PER DISPATCH
through the axon tunnel, and that cost scales with mesh size (~100 us
fixed + ~15-30 us per core: measured sustained slopes for a trivial
NEFF are 99/92/116/233 us at 1/2/4/8 cores), while the on-device time
scales down with more cores. 8 cores put the device at ~72 us but
dispatch at ~233 us; 4 cores put dispatch at ~116 us and the device at
~75 us (each core shares ALL key-side work -- k features, k trig, the
values load -- across its four query blocks). The dispatch cost also
grows with total buffer bytes, which is why the packed input ships as
bf16 (2.9 MB instead of 5.8 MB; everything is consumed as bf16
on-chip anyway).

Algorithm (sine decomposition of additive attention):

tanh(a+b) is separable through the angle-addition identity. Fit
tanh(x) ~ sum_r c_r sin(w_r x) (weighted least squares, R=7, wmax=3.0,
core max err ~8e-3 over the feature-sum range; softmax and the w_v
weighting absorb most of it -- measured output rel err is 2.7e-3 vs
the 2e-2 gate), then

  scores[q,k] = sum_h w_h tanh(qf_hq + kf_hk)
             = sum_r [ (c_r w_h sin(w_r qf)) . cos(w_r kf)
                     + (c_r w_h cos(w_r qf)) . sin(w_r kf) ]   (contract h)

i.e. 2R=14 accumulating 128-contraction matmuls on the tensor engine
instead of 268M scalar-engine tanh evaluations (~300 us/core direct).
The ACT Sin table is only accurate on [-pi, pi], so arguments are
range-reduced exactly, with both sides of the problem fused into wide
per-r ops over the combined [qf | kf] feature tile (see the inline
comments in _build_sine for the exact op chain).

IMPORTANT: no GPSIMD (Q7) instructions anywhere -- each dispatch of a
NEFF containing GPSIMD ops costs ~300-400 us of extra per-dispatch
host/runtime overhead under axon, dwarfing the on-device time. The
trig chains are balanced across DVE and ACT only. sin/cos tiles are
bf16 (the c_r*w_v weighting is folded into the q-side, keeping
per-term magnitudes small). Inputs arrive as ONE packed DRAM
parameter (fewer PJRT buffer binds per dispatch).

Scores are computed in [q, k] orientation (one [128, 1024] 2-bank PSUM
region per 128-query block, a single accumulation group, so plain
matmul start/stop works with no dummy zeroing). Softmax needs no
max-subtraction (scores are O(1) by construction: w_v has variance
1/H); exp's accum_out gives the denominator for free, and the attn
tile is xbar-DMA-transposed into the stationary operand of the attn@V
matmuls. Walrus's s3d3_mm ISA check caps a matmul's moving fmap at 512
elements and the AP partition stride at 16 KiB/row -- the score
matmuls are split into 512-wide halves and the per-r sin/cos tiles
stay separate for these reasons.

Dispatch-path notes (the dominant cost): the runner compiles under
bass2jax._fast_dispatch_active(True) so _bass_exec_p declares no
effect and calls take jax's C++ pjit fast path (the Python
effects/token path costs ~0.3-0.7 ms/call more); it returns the raw
Compiled rather than the FastDispatchCompiled wrapper (whose per-call
Python re-registration of every output shard costs ~0.1-0.2 ms); and
it passes no zero-filled output operands (on the exec lowering path
the NEFF binds only BIR ExternalInputs, and this kernel DMA-writes
every element of out). partition_id is disabled (unused input).
"""


import os
import ml_dtypes
import numpy as np

import concourse.bass as bass
import concourse.mybir as mybir
import concourse.tile as tile
from concourse import bacc
from concourse.bass_utils import run_bass_kernel_spmd
from concourse.masks import make_identity

B, Q, K, D, H = 4, 512, 1024, 512, 128
N_CORES = 4
QSH = Q                         # 512 query rows per core (one batch/core)
QH = 256                        # query rows per PSUM score pass
NDC = D // 128                  # 4 contraction chunks
NKC = K // 128                  # 8 key chunks
NQB = QSH // 128                # 4 query blocks per core

F32 = mybir.dt.float32
BF16 = mybir.dt.bfloat16
EXP = mybir.ActivationFunctionType.Exp
SIN = mybir.ActivationFunctionType.Sin
COPY = mybir.ActivationFunctionType.Copy
TS = mybir.AluOpType

MAGIC = 12582912.0              # 1.5 * 2**23: fp32 add forces round-to-int
TWO_PI = float(2.0 * np.pi)

LAST_EXEC_NS = None
_NC_CACHE = {}


R_SINE = 7
WMAX = 3.0

def _fit_sine(R=R_SINE, wmax=WMAX, L=7.5, sigma=2.8):
    """Least-squares fit tanh(x) ~ sum_r c_r sin(w_r x) on [-L, L]."""
    ws = np.linspace(wmax / R * 0.5, wmax, R)
    xs = np.linspace(-L, L, 4001)
    wt = np.exp(-xs ** 2 / (2 * sigma ** 2)) + 1e-3
    A = np.sin(np.outer(xs, ws))
    Wt = np.sqrt(wt)[:, None]
    c, *_ = np.linalg.lstsq(A * Wt, np.tanh(xs) * Wt[:, 0], rcond=None)
    return [float(w) for w in ws], [float(v) for v in c]


# Packed single-input layout (one NEFF parameter instead of six: fewer PJRT
# buffer binds per dispatch, which dominates the per-call overhead under axon).
OFF_Q = 0
OFF_K = OFF_Q + QSH * D          # 262144
OFF_V = OFF_K + K * D            # 786432
OFF_WQ = OFF_V + K * D           # 1310720
OFF_WK = OFF_WQ + D * H          # 1376256
OFF_WV = OFF_WK + D * H          # 1441792
NPACK = OFF_WV + H               # 1441920


def _declare_io(nc):
    # The packed input is BF16: every tensor is consumed as bf16 on-chip
    # anyway (features, trig, attn@V), so shipping bf16 halves the input
    # DMA bus time AND lets the xbar DMA-transposes read q/k straight from
    # DRAM with no staging tiles, no casts, and no queue dependencies.
    # Only w_v is precision-sensitive enough to notice, and its bf16
    # rounding adds well under 1e-3 output rel err.
    xin = nc.declare_dram_parameter("xin", [NPACK], BF16, isOutput=False)
    # Output is also bf16 (kernel() upconverts to f32 on the host): the
    # per-dispatch tunnel cost scales with buffer bytes, and the output
    # values' bf16 rounding (<=0.4% per element) fits the error budget.
    out_ext = nc.declare_dram_parameter("out", [QSH, D], BF16, isOutput=True)
    q_ext = xin[OFF_Q:OFF_K].rearrange("(q d) -> q d", d=D)
    k_ext = xin[OFF_K:OFF_V].rearrange("(k d) -> k d", d=D)
    v_ext = xin[OFF_V:OFF_WQ].rearrange("(k d) -> k d", d=D)
    wq_ext = xin[OFF_WQ:OFF_WK].rearrange("(d h) -> d h", h=H)
    wk_ext = xin[OFF_WK:OFF_WV].rearrange("(d h) -> d h", h=H)
    wv_ext = xin[OFF_WV:NPACK].rearrange("(h o) -> h o", o=1)
    return q_ext, k_ext, v_ext, wq_ext, wk_ext, wv_ext, out_ext


def _preamble(nc, tc, const, work, feat, q_ext, k_ext, v_ext, wq_ext, wk_ext,
              wv_ext):
    """Load + transpose inputs, feature matmuls.

    The packed input is bf16, so the xbar DMA-transposes read q and k
    STRAIGHT FROM DRAM: no f32 staging tiles, no casts, and -- because a
    DRAM source has no producer -- no semaphore waits on the in-order SP
    queue. A DRAM [rows, 512] transpose lands as [128, NDC, rows]
    (logical row d -> partition d % 128, chunk d // 128), so the feature
    matmuls read contiguous [128, rows] moving fmaps. k is transposed in
    256-row quarters so each quarter's kf matmuls overlap the next
    quarter's transfer; each quarter accumulates in its own single-bank
    PSUM tile. v loads directly as the bf16 attn@V operand. Every
    per-DMA instruction costs ~0.6-1.2 us of sequencer issue + ~0.9 us
    of completion semaphore, so transfers are batched (8 DMA instructions
    total for 2.9 MB).

    Returns (fq [H, QSH+K] f32 ([qf | kf]), v_b [128, NKC, D] bf16,
    wv_f [H,1] f32)."""
    wq_b = const.tile([128, NDC, H], BF16)
    wk_b = const.tile([128, NDC, H], BF16)
    nc.sync.dma_start(out=wk_b, in_=wk_ext.rearrange("(c p) h -> p c h", p=128))
    nc.sync.dma_start(out=wq_b, in_=wq_ext.rearrange("(c p) h -> p c h", p=128))

    wv_b = const.tile([H, 1], BF16)
    nc.sync.dma_start(out=wv_b, in_=wv_ext[:])
    wv_f = const.tile([H, 1], F32)
    nc.vector.tensor_copy(wv_f, wv_b)

    qTb = work.tile([128, NDC, QSH], BF16)
    nc.sync.dma_start_transpose(out=qTb[:], in_=q_ext)
    kTb = work.tile([128, 4, NDC, 256], BF16)
    for qd in range(4):
        nc.sync.dma_start_transpose(out=kTb[:, qd],
                                    in_=k_ext[qd * 256:(qd + 1) * 256, :])

    # fq = [qf | kf] in ONE tile so the per-r trig argument prep can run as
    # single wide DVE ops over both sides at once.
    fq = feat.tile([H, QSH + K], F32)
    qf_sb = fq[:, :QSH]
    kf_sb = fq[:, QSH:]
    with tc.tile_pool(name="pre_ps", bufs=4, space="PSUM") as pre_ps:
        qf_ps = pre_ps.tile([H, QSH], F32, tag="fps")
        for dc in range(NDC):
            nc.tensor.matmul(qf_ps, wq_b[:, dc, :], qTb[:, dc, :],
                             start=(dc == 0), stop=(dc == NDC - 1))
        nc.vector.tensor_copy(qf_sb, qf_ps)

        for qd in range(4):
            kf_q = pre_ps.tile([H, 256], F32, tag="kfq")
            for dc in range(NDC):
                nc.tensor.matmul(kf_q, wk_b[:, dc, :], kTb[:, qd, dc, :],
                                 start=(dc == 0), stop=(dc == NDC - 1))
            dst = kf_sb[:, qd * 256:(qd + 1) * 256]
            (nc.vector.tensor_copy(dst, kf_q) if qd % 2 == 0
             else nc.scalar.copy(dst, kf_q))

    v_b = feat.tile([128, NKC, D], BF16)
    tc.tile_set_cur_wait(0.05)   # keep values off the keys->kf critical path
    nc.sync.dma_start(out=v_b,
                      in_=v_ext.rearrange("(c p) d -> p c d", p=128))
    tc.tile_set_cur_wait(0)

    return fq, v_b, wv_f


def _build_sine():
    ws, cs = _fit_sine()
    R = len(ws)
    nc = bacc.Bacc(enable_partition_id=False)
    q_ext, k_ext, v_ext, wq_ext, wk_ext, wv_ext, out_ext = _declare_io(nc)

    with tile.TileContext(nc) as tc:
        with tc.tile_pool(name="const", bufs=1) as const, \
             tc.tile_pool(name="feat", bufs=1) as feat, \
             tc.tile_pool(name="trig", bufs=4) as trig, \
             tc.tile_pool(name="oloop", bufs=2) as oloop:

            # The staging tiles (k_all/q_all/kT/qT/v_stage, ~64 KiB/
            # partition) live in their own pool that closes after the
            # preamble, freeing the space for the deeper (bufs=3) trig
            # pipeline.
            with tc.tile_pool(name="featpre", bufs=1) as featpre:
                fq, v_b, wv_f = _preamble(
                    nc, tc, const, featpre, feat, q_ext, k_ext, v_ext,
                    wq_ext, wk_ext, wv_ext)

            # per-r q-side coefficient vectors: wc[:, r] = c_r * w_v
            wc = const.tile([H, R], F32)
            for r in range(R):
                nc.vector.tensor_scalar_mul(wc[:, r:r + 1], wv_f, float(cs[r]))

            W2 = QSH + K                 # one trig block: [q(512) | k(1024)]
            # SCB[r] = [wc*sin_q(512) | sin_k(1024) |
            #           wc*cos_q(512) | cos_k(1024)]
            # One tile PER r (not [H, R, 2*W2]): the PE matmul AP encodes a
            # per-partition stride that maxes out at 16 KiB, and the fused
            # tile's 24 KiB row fails walrus's s3d3_mm ISA check.
            SCB = [feat.tile([H, 2 * W2], BF16, name=f"scb{r}")
                   for r in range(R)]

            # Per r, the range reduction is 3 wide DVE ops + 1 ACT round +
            # ONE double-wide ACT Sin over the combined [qf | kf] block:
            #   t   = x * (w/2pi)                  (DVE tensor_scalar)
            #   a   = fl(t + 1.5*2^23)             (ACT Copy, float bias:
            #                                       the fp32 store rounds t
            #                                       to the nearest integer)
            #   e_s = (a - MAGIC) - t              (DVE scalar_tensor_tensor,
            #                                       exact) = round(t) - t
            #   e_c = wrap(e_s - 1/4)              (custom-DVE range wrap
            #                                       into [-1/2, 1/2])
            #   sin/cos(w x) = Sin(-2pi * e)       (ONE ACT Sin over the
            #                                       contiguous [e_s | e_c]
            #                                       tile; table is accurate
            #                                       on [-pi, pi])
            # The q-side c_r*w_v weighting is applied in place on the bf16
            # sin/cos q slices (2 narrow DVE muls). No GPSIMD (Q7) anywhere:
            # those cost ~300us of per-dispatch host overhead under axon.
            for r in range(R):
                w2p = float(ws[r] / TWO_PI)
                wcol = wc[:, r:r + 1]
                t_t = trig.tile([H, W2], F32, tag="t")
                nc.vector.tensor_scalar(t_t, fq, w2p, None, TS.mult)
                a_t = trig.tile([H, W2], F32, tag="a")
                nc.scalar.activation(out=a_t, in_=t_t, func=COPY, bias=MAGIC)
                arg = trig.tile([H, 2 * W2], F32, tag="arg")
                nc.vector.scalar_tensor_tensor(arg[:, :W2], a_t, MAGIC, t_t,
                                               TS.subtract, TS.subtract)
                nc.vector.add_range_wrap(arg[:, W2:], arg[:, :W2],
                                         -0.25, 0.5, 1.0)
                nc.scalar.activation(out=SCB[r][:], in_=arg,
                                     func=SIN, scale=-TWO_PI)
                nc.vector.tensor_scalar_mul(SCB[r][:, 0:QSH],
                                            SCB[r][:, 0:QSH], wcol)
                nc.vector.tensor_scalar_mul(SCB[r][:, W2:W2 + QSH],
                                            SCB[r][:, W2:W2 + QSH], wcol)


            # Scores in [q, k] orientation: per 128-query block, stationary
            # = QS/QC q-chunk [H, 128] and moving = the FULL 1024-wide
            # KS/KC row, so one PSUM pass is 2R=16 matmuls of 1024 moving
            # cols instead of 128 matmuls of 256 cols. This unloads the PE
            # SEQUENCER (Ldweights+Matmult issue was the critical path at
            # 256 score matmuls). Each [128, 1024] f32 region is exactly 2
            # PSUM banks used by a single accumulation group, so plain
            # start/stop works -- no dummy-zero matmuls.
            #
            # attn is then transposed for the attn@V matmuls by ONE xbar
            # DMA-transpose per block (64 16x128 tiles, ~1 us on the idle
            # DMA engines) instead of 8 PE transposes + 8 PSUM copies. The
            # xbar writes logical row r of attn^T to attnT[r % 128,
            # r // 128, :] (hardware-verified), i.e. k-chunk c holds k rows
            # {c*128+p} -- the natural chunk layout v_b is loaded in.
            # Score matmuls for block qb+1 are issued BEFORE block qb's
            # transpose/AV so the PE never stalls on qb's exp.
            o_all = feat.tile([128, NQB, D], BF16)
            with tc.tile_pool(name="psqk", bufs=3, space="PSUM") as psqk, \
                 tc.tile_pool(name="ps", bufs=2, space="PSUM") as ps:
                qsc = [None] * NQB

                def issue_scores(qb):
                    qs_sl = slice(qb * 128, (qb + 1) * 128)
                    qc_sl = slice(W2 + qb * 128, W2 + (qb + 1) * 128)
                    ks_sl = slice(QSH, W2)
                    kc_sl = slice(W2 + QSH, 2 * W2)
                    qsc[qb] = psqk.tile([128, K], F32, tag="qsc",
                                        name=f"qsc{qb}")
                    # moving operands are split into 512-wide halves: a
                    # 1024-element moving fmap fails walrus's s3d3_mm ISA
                    # check. The stationary is identical for both halves, so
                    # the second matmul skips its Ldweights.
                    for r in range(R):
                        for hf in range(2):
                            osl = slice(hf * 512, (hf + 1) * 512)
                            nc.tensor.matmul(
                                qsc[qb][:, osl], SCB[r][:, qs_sl],
                                SCB[r][:, kc_sl][:, osl],
                                start=(r == 0), stop=False,
                                skip_group_check=True)
                        for hf in range(2):
                            osl = slice(hf * 512, (hf + 1) * 512)
                            nc.tensor.matmul(
                                qsc[qb][:, osl], SCB[r][:, qc_sl],
                                SCB[r][:, ks_sl][:, osl],
                                start=False, stop=(r == R - 1),
                                skip_group_check=True)

                issue_scores(0)
                issue_scores(1)
                for qb in range(NQB):
                    attnQ = oloop.tile([128, K], BF16, tag="attnQ")
                    d_sb = oloop.tile([128, 1], F32, tag="dsb")
                    # accum_out gives the softmax denominator for free: in
                    # the [q, k] orientation the activation's per-partition
                    # output sum IS sum_k exp(score[q, k]).
                    nc.scalar.activation(out=attnQ, in_=qsc[qb], func=EXP,
                                         accum_out=d_sb)
                    if qb + 2 < NQB:
                        issue_scores(qb + 2)
                    attnT = oloop.tile([128, NKC, 128], BF16, tag="attnT")
                    nc.sync.dma_start_transpose(out=attnT[:], in_=attnQ[:])
                    o_ps = ps.tile([128, D], F32, tag="ops")
                    for kc in range(NKC):
                        nc.tensor.matmul(o_ps, attnT[:, kc, :], v_b[:, kc, :],
                                         start=(kc == 0), stop=(kc == NKC - 1))
                    recip = oloop.tile([128, 1], F32, tag="recip")
                    nc.vector.reciprocal(recip, d_sb)
                    nc.vector.tensor_scalar_mul(o_all[:, qb, :], o_ps, recip)
                    if qb == 1:
                        # first output half leaves while qb2/qb3 compute
                        nc.sync.dma_start(
                            out=out_ext.rearrange(
                                "(t p) d -> p t d", p=128)[:, 0:2, :],
                            in_=o_all[:, 0:2, :])
                nc.sync.dma_start(
                    out=out_ext.rearrange("(t p) d -> p t d", p=128)[:, 2:4, :],
                    in_=o_all[:, 2:4, :])
    nc.compile()
    return nc


def _get_nc():
    if "sine" not in _NC_CACHE:
        _NC_CACHE["sine"] = _build_sine()
    return _NC_CACHE["sine"]


def make_in_maps(queries, keys, values, W_q, W_k, w_v):
    bf16 = ml_dtypes.bfloat16
    queries = np.asarray(queries).astype(bf16)
    keys = np.asarray(keys).astype(bf16)
    values = np.asarray(values).astype(bf16)
    W_q = np.asarray(W_q).astype(bf16).ravel()
    W_k = np.asarray(W_k).astype(bf16).ravel()
    w_v = np.asarray(w_v).astype(bf16).ravel()
    in_maps = []
    for c in range(N_CORES):
        buf = np.empty(NPACK, bf16)
        buf[OFF_Q:OFF_K] = queries[c].ravel()
        buf[OFF_K:OFF_V] = keys[c].ravel()
        buf[OFF_V:OFF_WQ] = values[c].ravel()
        buf[OFF_WQ:OFF_WK] = W_q
        buf[OFF_WK:OFF_WV] = W_k
        buf[OFF_WV:NPACK] = w_v
        in_maps.append({"xin": buf})
    return in_maps


_RUNNER_CACHE = {}


def _get_runner(nc):
    """Persistent compiled shard_map runner for nc (compiled once/process).

    Two dispatch-path choices matter for the marginal per-call cost under
    axon (the per-dispatch host overhead dominates on-device time):

    * compile under bass2jax._fast_dispatch_active(True): _bass_exec_p then
      declares no effect, so calls take jax's C++ pjit fast path instead of
      the Python effects/token dispatch (~0.3-0.7 ms/call cheaper).
    * return the raw Compiled, NOT FastDispatchCompiled: the safety-net
      wrapper re-registers every output shard in runtime_tokens on every
      call (a Python loop over the shards, ~0.1-0.2 ms/call). kernel()
      reads its outputs immediately, so device errors surface regardless.
    * no zero-filled output operands: on the exec lowering path the NEFF
      binds only BIR ExternalInputs (the "out" zeros param has no NEFF
      tensor and is ignored), and this kernel DMA-writes every element of
      out, so PJRT's uninitialized result allocation is fine. Dropping
      them saves one buffer bind per core per call.
    """
    if id(nc) in _RUNNER_CACHE:
        return _RUNNER_CACHE[id(nc)]
    import jax
    from jax.sharding import Mesh, NamedSharding, PartitionSpec
    from jax.experimental.shard_map import shard_map
    from concourse import bass2jax

    bass2jax.install_neuronx_cc_hook()
    partition_name = (nc.partition_id_tensor.name
                      if nc.partition_id_tensor else None)
    in_names, in_shapes, out_names, out_avals = [], [], [], []
    for alloc in nc.m.functions[0].allocations:
        if not isinstance(alloc, mybir.MemoryLocationSet):
            continue
        name = alloc.memorylocations[0].name
        if alloc.kind == "ExternalInput":
            if name != partition_name:
                in_names.append(name)
                in_shapes.append(
                    (tuple(alloc.tensor_shape), mybir.dt.np(alloc.dtype)))
        elif alloc.kind == "ExternalOutput":
            out_names.append(name)
            shape = tuple(alloc.tensor_shape)
            dtype = mybir.dt.np(alloc.dtype)
            out_avals.append(jax.core.ShapedArray(shape, dtype))
    all_in_names = list(in_names)
    if partition_name is not None:
        all_in_names.append(partition_name)

    def _body(*args):
        operands = list(args)
        if partition_name is not None:
            operands.append(bass2jax.partition_id_tensor())
        outs = bass2jax._bass_exec_p.bind(
            *operands,
            out_avals=tuple(out_avals),
            in_names=tuple(all_in_names),
            out_names=tuple(out_names),
            lowering_input_output_aliases=(),
            sim_require_finite=True,
            sim_require_nnan=True,
            nc=nc,
        )
        return tuple(outs)

    devices = jax.devices()[:N_CORES]
    mesh = Mesh(np.asarray(devices), ("core",))
    nio = len(in_names)
    sharding = NamedSharding(mesh, PartitionSpec("core"))
    fast_ctx = getattr(bass2jax, "_fast_dispatch_active", None)
    import contextlib
    with (fast_ctx(True) if fast_ctx is not None
          else contextlib.nullcontext()):
        f = jax.jit(
            shard_map(_body, mesh=mesh,
                      in_specs=(PartitionSpec("core"),) * nio,
                      out_specs=(PartitionSpec("core"),) * len(out_names),
                      check_rep=False),
            keep_unused=True,
        )
        dummy_in = [
            jax.ShapeDtypeStruct((N_CORES * shape[0], *shape[1:]), dtype,
                                 sharding=sharding)
            for shape, dtype in in_shapes
        ]
        fc = f.lower(*dummy_in).compile()
    runner = (fc, in_names, out_names, out_avals, sharding)
    _RUNNER_CACHE[id(nc)] = runner
    return runner


def kernel(queries, keys, values, W_q, W_k, w_v):
    import jax
    nc = _get_nc()
    in_maps = make_in_maps(queries, keys, values, W_q, W_k, w_v)
    try:
        fc, in_names, out_names, out_avals, sharding = _get_runner(nc)
        concat_in = [
            np.concatenate([in_maps[c][name] for c in range(N_CORES)], axis=0)
            for name in in_names
        ]
        args = [jax.device_put(a, sharding) for a in concat_in]
        out_arrs = fc(*args)
        results = [
            {name: np.asarray(out_arrs[i]).reshape(
                N_CORES, *out_avals[i].shape)[c]
             for i, name in enumerate(out_names)}
            for c in range(N_CORES)
        ]
    except Exception:
        res = run_bass_kernel_spmd(nc, in_maps, core_ids=list(range(N_CORES)))
        results = res.results

    out = np.empty((B, Q, D), dtype=np.float32)
    for c in range(N_CORES):
        out[c] = results[c]["out"].astype(np.float32)
    return out
